# revision 3
# baseline (speedup 1.0000x reference)
"""Bass/Trainium2 kernel for nn_AvgPoolBackbone (segment_reduce), packed.

Computes, for each batch row b of x [B, S, D]:
    eff = S if idx[b] == -1 else idx[b]
    out[b] = mean(x[b, :eff], axis=0)   (zeros when eff <= 0)

Strategy
--------
Rows past eff[b] contribute nothing, so the kernel never reads them.
The host packs each batch's valid prefix (rounded up to 16-row groups;
the tail rows of the last, partial group are left zero) into one dense
stream and splits it evenly across the 8 cores at group granularity
(batches may straddle a core boundary; their two partial sums are added
on the host).  Per core that is ~18 MB of HBM traffic instead of
33.5 MB — the kernel is purely DMA-bound, so this is the big lever.

Device layout: the per-core stream is a sequence of 2 MiB chunks
[128 partitions x 16 rows x 256], partition p holding 16 consecutive
stream rows (one 16 KiB contiguous DRAM run per partition).  Because
the host zeroes invalid rows, every partition's 16 rows share one
weight (1/eff of its batch, or 0 for padding), so the whole reduction
collapses onto TensorE: per chunk, 8 float32r matmuls

    psum[NB, 512] += selw_t.T @ x_t[:, j*512:(j+1)*512]

with selw[p, slot] = 1/eff (the weighted partition->batch-slot selector
built on the host) as the shared stationary matrix.  Each matmul folds
two row-slices at once (moving free dim 512 = 2 x D); all matmuls of
all chunks but the last accumulate into one [NB, 512] PSUM group
whose folded [NB, 256] partial ships while the final chunk is still
streaming; the last chunk accumulates into a second PSUM group, so the
post-stream critical path is one small fold + DMA (the host adds the
two partials).  DVE/ACT/GpSimd are otherwise idle; the DMA x-stream
never waits on compute (every piece has its own SBUF slot).

float32r matmuls are single-pass (reduced-precision fp32, rel err
~1e-5 here vs the fp32 reference, tolerance is 2e-2).
"""

import numpy as np

import concourse.bass as bass
import concourse.tile as tile
from concourse import bacc, mybir
from concourse import bass_utils

F32 = mybir.dt.float32
F32R = mybir.dt.float32r

# Problem config (hardcoded per the harness contract).
B, S, D = 128, 2048, 256
N_CORES = 8
P = 128            # SBUF partitions
GRP = 16           # stream rows per group (one partition's rows per chunk)
CHUNK_G = 128      # groups per chunk (= 2048 rows = 2 MiB)
MMF = 512          # moving free dim per matmul (2 slices of D)


def plan_shards(start_padding_indices):
    idx = np.asarray(start_padding_indices).astype(np.int64).reshape(-1)
    eff = np.where(idx == -1, S, np.clip(idx, 0, S)).astype(np.int64)
    gb = (eff + GRP - 1) // GRP          # 16-row groups per batch
    g_total = int(gb.sum())
    gt = max(-(-g_total // N_CORES), 1)  # groups per core
    cum = np.concatenate([[0], np.cumsum(gb)])
    cores = []
    for c in range(N_CORES):
        lo, hi = c * gt, (c + 1) * gt
        segs = []   # (batch, first group within batch, n groups, dst group)
        for b in range(B):
            s0, s1 = int(cum[b]), int(cum[b + 1])
            o0, o1 = max(s0, lo), min(s1, hi)
            if o0 < o1:
                segs.append((b, o0 - s0, o1 - o0, o0 - lo))
        cores.append(segs)
    nb = max(max((len(s) for s in cores), default=1), 1)
    return eff, gt, nb, cores


def make_host_inputs(x, eff, gt, nb, cores):
    x = np.asarray(x, dtype=np.float32)
    t_chunks = -(-gt // CHUNK_G)
    gt_pad = t_chunks * CHUNK_G
    in_maps, slot_maps = [], []
    for segs in cores:
        xc = np.zeros((gt * GRP, D), dtype=np.float32)
        sv = np.zeros((gt_pad, nb), dtype=np.float32)
        slots = []
        for i, (b, g0, gc, dst) in enumerate(segs):
            slots.append(b)
            r0, r1 = g0 * GRP, (g0 + gc) * GRP
            r1v = min(r1, int(eff[b]))   # only valid rows; group tail stays 0
            if r1v > r0:
                xc[dst * GRP : dst * GRP + (r1v - r0)] = x[b, r0:r1v]
            sv[dst : dst + gc, i] = 1.0 / max(float(eff[b]), 1.0)
        # [gt_pad, nb] -> [T, 128, nb] -> [128, T*nb]  (partition-major)
        st = sv.reshape(t_chunks, CHUNK_G, nb).transpose(1, 0, 2)
        in_maps.append(
            {
                "x": np.ascontiguousarray(xc.reshape(gt, GRP * D)),
                "selw": np.ascontiguousarray(st.reshape(CHUNK_G, t_chunks * nb)),
            }
        )
        slot_maps.append(slots)
    return in_maps, slot_maps


def build_kernel(gt, nb):
    """Single-core Bass module (same NEFF on all cores)."""
    t_chunks = -(-gt // CHUNK_G)
    n_full = gt // CHUNK_G
    g_r = gt % CHUNK_G
    n_mmpc = (GRP * D) // MMF            # matmuls per chunk (8)
    total_mm = t_chunks * n_mmpc

    nc = bacc.Bacc("TRN2", target_bir_lowering=False, debug=False)
    x = nc.dram_tensor("x", (gt, GRP * D), F32R, kind="ExternalInput")
    selw = nc.dram_tensor("selw", (P, t_chunks * nb), F32R, kind="ExternalInput")
    # two partial outputs: "a" covers every chunk but the last and ships
    # while the last chunk is still streaming; "b" is only the last chunk,
    # so the post-stream tail is one small combine + DMA.  The host adds
    # them.
    out_a = nc.dram_tensor("out_a", (nb, D), F32, kind="ExternalOutput")
    out_b = nc.dram_tensor("out_b", (nb, D), F32, kind="ExternalOutput")

    # Process the (short) remainder chunk first so the stream ends on a
    # full, engine-balanced chunk; split every chunk's DMA into pieces
    # (halves; quarters for the final chunk) with one SBUF tile each so
    # the matmuls chase the stream at sub-chunk granularity and the
    # post-stream tail is just a quarter-chunk of matmuls.
    # remainder chunk second: not last (it would leave the stream tail
    # partition-imbalanced) and not first (its partial partition set
    # leaves some DMA engines idle at stream start)
    order = list(range(n_full))
    if g_r:
        order.insert(min(1, len(order)), t_chunks - 1)
    pieces = []
    for oi, t in enumerate(order):
        pc = g_r if (g_r and t == t_chunks - 1) else CHUNK_G
        ksp = (
            (0, 2, 4, 6, 8, 10, 12, 14, GRP)
            if oi == len(order) - 1
            else (0, 8, GRP)
        )
        for k0, k1 in zip(ksp[:-1], ksp[1:]):
            pieces.append((t, pc, k0, k1))

    with tile.TileContext(nc) as tc:
        with (
            tc.tile_pool(name="xp", bufs=1) as xp,
            tc.tile_pool(name="mp", bufs=1) as mp,
            tc.tile_pool(name="op", bufs=1) as op,
            tc.tile_pool(name="ps", bufs=1, space=bass.MemorySpace.PSUM) as ps,
        ):
            s_t = mp.tile([P, t_chunks * nb], F32R, tag="selw")
            # selector load on the scalar HWDGE ring so the sync ring's
            # x stream starts immediately
            nc.scalar.dma_start(s_t[:], selw.ap())
            o_ta = op.tile([nb, D], F32, tag="oa")
            o_tb = op.tile([nb, D], F32, tag="ob")
            ps_a = ps.tile([nb, MMF], F32, tag="psa")
            ps_b = ps.tile([nb, MMF], F32, tag="psb")
            xv = x.ap()
            last_t = order[-1]
            n_mmpc = GRP * D // MMF
            mm_a_total = (t_chunks - 1) * n_mmpc
            mm_a = mm_b = 0

            def combine_and_ship(o_t, ps_t, eng, out_dram):
                eng_names = {"v"}
                nc.vector.tensor_scalar_mul(o_t[:], ps_t[:, :D], 1.0)
                nc.vector.scalar_tensor_tensor(
                    o_t[:],
                    ps_t[:, D:],
                    1.0,
                    o_t[:],
                    mybir.AluOpType.mult,
                    mybir.AluOpType.add,
                )
                eng.dma_start(out_dram.ap(), o_t[:])

            for idx, (t, pc, k0, k1) in enumerate(pieces):
                if t == last_t and mm_b == 0 and mm_a_total > 0:
                    # every non-final chunk is folded: ship partial "a" now,
                    # overlapping the final chunk's stream and matmuls
                    combine_and_ship(o_ta, ps_a, nc.scalar, out_a)
                x_t = xp.tile(
                    [pc, (k1 - k0) * D], F32R, tag=f"x{idx}", name=f"x{idx}"
                )
                nc.sync.dma_start(
                    x_t[:],
                    xv[t * CHUNK_G : t * CHUNK_G + pc, k0 * D : k1 * D],
                )
                for jl in range((k1 - k0) * D // MMF):
                    if t == last_t:
                        nc.tensor.matmul(
                            ps_b[:],
                            s_t[:pc, t * nb : (t + 1) * nb],
                            x_t[:, jl * MMF : (jl + 1) * MMF],
                            start=(mm_b == 0),
                            stop=(mm_b == n_mmpc - 1),
                        )
                        mm_b += 1
                    else:
                        nc.tensor.matmul(
                            ps_a[:],
                            s_t[:pc, t * nb : (t + 1) * nb],
                            x_t[:, jl * MMF : (jl + 1) * MMF],
                            start=(mm_a == 0),
                            stop=(mm_a == mm_a_total - 1),
                        )
                        mm_a += 1
            combine_and_ship(o_tb, ps_b, nc.sync, out_b)

    nc.compile()
    return nc


_CACHED = {}


def _get_nc(gt, nb):
    key = (gt, nb)
    if key not in _CACHED:
        _CACHED[key] = build_kernel(gt, nb)
    return _CACHED[key]


def run(x, start_padding_indices, trace=False):
    eff, gt, nb, cores = plan_shards(start_padding_indices)
    in_maps, slot_maps = make_host_inputs(x, eff, gt, nb, cores)
    nc = _get_nc(gt, nb)
    res = bass_utils.run_bass_kernel_spmd(
        nc, in_maps, core_ids=list(range(N_CORES)), trace=trace
    )
    out_full = np.zeros((B, D), dtype=np.float32)
    t_chunks = -(-gt // CHUNK_G)
    for c in range(N_CORES):
        oc = res.results[c]["out_b"].reshape(nb, D)
        if t_chunks > 1:
            oc = oc + res.results[c]["out_a"].reshape(nb, D)
        for i, b in enumerate(slot_maps[c]):
            out_full[b] += oc[i]
    return out_full, res


def kernel(x, start_padding_indices):
    out, _ = run(x, start_padding_indices, trace=False)
    return out


# revision 4
# speedup vs baseline: 1.0550x; 1.0550x over previous
"""Bass/Trainium2 kernel for nn_AvgPoolBackbone (segment_reduce), packed.

Computes, for each batch row b of x [B, S, D]:
    eff = S if idx[b] == -1 else idx[b]
    out[b] = mean(x[b, :eff], axis=0)   (zeros when eff <= 0)

Strategy
--------
Rows past eff[b] contribute nothing, so the kernel never reads them.
The host packs each batch's valid prefix (rounded up to 16-row groups;
the tail rows of the last, partial group are left zero) into one dense
stream and splits it evenly across the 8 cores at group granularity
(batches may straddle a core boundary; their two partial sums are added
on the host).  Per core that is ~18 MB of HBM traffic instead of
33.5 MB — the kernel is purely DMA-bound, so this is the big lever.

Device layout: the per-core stream is a sequence of 2 MiB chunks
[128 partitions x 16 rows x 256], partition p holding 16 consecutive
stream rows (one 16 KiB contiguous DRAM run per partition).  Because
the host zeroes invalid rows, every partition's 16 rows share one
weight (1/eff of its batch, or 0 for padding), so the whole reduction
collapses onto TensorE: per chunk, 8 float32r matmuls

    psum[NB, 512] += selw_t.T @ x_t[:, j*512:(j+1)*512]

with selw[p, slot] = 1/eff (the weighted partition->batch-slot selector
built on the host) as the shared stationary matrix.  Each matmul folds
two row-slices at once (moving free dim 512 = 2 x D); all matmuls of
all chunks but the last accumulate into one [NB, 512] PSUM group
whose folded [NB, 256] partial ships while the final chunk is still
streaming; the last chunk accumulates into a second PSUM group, so the
post-stream critical path is one small fold + DMA (the host adds the
two partials).  DVE/ACT/GpSimd are otherwise idle; the DMA x-stream
never waits on compute (every piece has its own SBUF slot).

float32r matmuls are single-pass (reduced-precision fp32, rel err
~1e-5 here vs the fp32 reference, tolerance is 2e-2).
"""

import numpy as np

import concourse.bass as bass
import concourse.tile as tile
from concourse import bacc, mybir
from concourse import bass_utils

F32 = mybir.dt.float32
F32R = mybir.dt.float32r

# Problem config (hardcoded per the harness contract).
B, S, D = 128, 2048, 256
N_CORES = 8
P = 128            # SBUF partitions
GRP = 16           # stream rows per group (one partition's rows per chunk)
CHUNK_G = 128      # groups per chunk (= 2048 rows = 2 MiB)
MMF = 512          # moving free dim per matmul (2 slices of D)


def plan_shards(start_padding_indices):
    idx = np.asarray(start_padding_indices).astype(np.int64).reshape(-1)
    eff = np.where(idx == -1, S, np.clip(idx, 0, S)).astype(np.int64)
    gb = (eff + GRP - 1) // GRP          # 16-row groups per batch
    g_total = int(gb.sum())
    gt = max(-(-g_total // N_CORES), 1)  # groups per core
    cum = np.concatenate([[0], np.cumsum(gb)])
    cores = []
    for c in range(N_CORES):
        lo, hi = c * gt, (c + 1) * gt
        segs = []   # (batch, first group within batch, n groups, dst group)
        for b in range(B):
            s0, s1 = int(cum[b]), int(cum[b + 1])
            o0, o1 = max(s0, lo), min(s1, hi)
            if o0 < o1:
                segs.append((b, o0 - s0, o1 - o0, o0 - lo))
        cores.append(segs)
    nb = max(max((len(s) for s in cores), default=1), 1)
    return eff, gt, nb, cores


def make_host_inputs(x, eff, gt, nb, cores):
    x = np.asarray(x, dtype=np.float32)
    t_chunks = -(-gt // CHUNK_G)
    gt_pad = t_chunks * CHUNK_G
    in_maps, slot_maps = [], []
    for segs in cores:
        xc = np.zeros((gt * GRP, D), dtype=np.float32)
        sv = np.zeros((gt_pad, nb), dtype=np.float32)
        slots = []
        for i, (b, g0, gc, dst) in enumerate(segs):
            slots.append(b)
            r0, r1 = g0 * GRP, (g0 + gc) * GRP
            r1v = min(r1, int(eff[b]))   # only valid rows; group tail stays 0
            if r1v > r0:
                xc[dst * GRP : dst * GRP + (r1v - r0)] = x[b, r0:r1v]
            sv[dst : dst + gc, i] = 1.0 / max(float(eff[b]), 1.0)
        # [gt_pad, nb] -> [T, 128, nb] -> [128, T*nb]  (partition-major)
        st = sv.reshape(t_chunks, CHUNK_G, nb).transpose(1, 0, 2)
        in_maps.append(
            {
                "x": np.ascontiguousarray(xc.reshape(gt, GRP * D)),
                "selw": np.ascontiguousarray(st.reshape(CHUNK_G, t_chunks * nb)),
            }
        )
        slot_maps.append(slots)
    return in_maps, slot_maps


def build_kernel(gt, nb):
    """Single-core Bass module (same NEFF on all cores)."""
    t_chunks = -(-gt // CHUNK_G)
    n_full = gt // CHUNK_G
    g_r = gt % CHUNK_G
    n_mmpc = (GRP * D) // MMF            # matmuls per chunk (8)
    total_mm = t_chunks * n_mmpc

    nc = bacc.Bacc("TRN2", target_bir_lowering=False, debug=False)
    x = nc.dram_tensor("x", (gt, GRP * D), F32R, kind="ExternalInput")
    selw = nc.dram_tensor("selw", (P, t_chunks * nb), F32R, kind="ExternalInput")
    # two partial outputs: "a" covers every chunk but the last and ships
    # while the last chunk is still streaming; "b" is only the last chunk,
    # so the post-stream tail is one small combine + DMA.  The host adds
    # them.
    out_a = nc.dram_tensor("out_a", (nb, D), F32, kind="ExternalOutput")
    out_b = nc.dram_tensor("out_b", (nb, D), F32, kind="ExternalOutput")

    # Process the (short) remainder chunk first so the stream ends on a
    # full, engine-balanced chunk; split every chunk's DMA into pieces
    # (halves; quarters for the final chunk) with one SBUF tile each so
    # the matmuls chase the stream at sub-chunk granularity and the
    # post-stream tail is just a quarter-chunk of matmuls.
    # remainder chunk second: not last (it would leave the stream tail
    # partition-imbalanced) and not first (its partial partition set
    # leaves some DMA engines idle at stream start)
    order = list(range(n_full))
    if g_r:
        order.insert(min(1, len(order)), t_chunks - 1)
    pieces = []
    for oi, t in enumerate(order):
        pc = g_r if (g_r and t == t_chunks - 1) else CHUNK_G
        ksp = (
            (0, 2, 4, 6, 8, 10, 12, 14, GRP)
            if oi == len(order) - 1
            else (0, 8, GRP)
        )
        for k0, k1 in zip(ksp[:-1], ksp[1:]):
            pieces.append((t, pc, k0, k1))

    with tile.TileContext(nc) as tc:
        with (
            tc.tile_pool(name="xp", bufs=1) as xp,
            tc.tile_pool(name="mp", bufs=1) as mp,
            tc.tile_pool(name="op", bufs=1) as op,
            tc.tile_pool(name="ps", bufs=1, space=bass.MemorySpace.PSUM) as ps,
        ):
            s_t = mp.tile([P, t_chunks * nb], F32R, tag="selw")
            # selector load on the scalar HWDGE ring so the sync ring's
            # x stream starts immediately
            nc.scalar.dma_start(s_t[:], selw.ap())
            o_ta = op.tile([nb, D], F32, tag="oa")
            o_tb = op.tile([nb, D], F32, tag="ob")
            ps_a = ps.tile([nb, MMF], F32, tag="psa")
            ps_b = ps.tile([nb, MMF], F32, tag="psb")
            xv = x.ap()
            last_t = order[-1]
            n_mmpc = GRP * D // MMF
            mm_a_total = (t_chunks - 1) * n_mmpc
            mm_a = mm_b = 0

            def combine_and_ship(o_t, ps_t, eng, out_dram):
                nc.vector.tensor_scalar_mul(o_t[:], ps_t[:, :D], 1.0)
                nc.vector.scalar_tensor_tensor(
                    o_t[:],
                    ps_t[:, D:],
                    1.0,
                    o_t[:],
                    mybir.AluOpType.mult,
                    mybir.AluOpType.add,
                )
                eng.dma_start(out_dram.ap(), o_t[:])

            for idx, (t, pc, k0, k1) in enumerate(pieces):
                if t == last_t and mm_b == 0 and mm_a_total > 0:
                    # every non-final chunk is folded: ship partial "a" now,
                    # overlapping the final chunk's stream and matmuls
                    combine_and_ship(o_ta, ps_a, nc.scalar, out_a)
                x_t = xp.tile(
                    [pc, (k1 - k0) * D], F32R, tag=f"x{idx}", name=f"x{idx}"
                )
                nc.sync.dma_start(
                    x_t[:],
                    xv[t * CHUNK_G : t * CHUNK_G + pc, k0 * D : k1 * D],
                )
                for jl in range((k1 - k0) * D // MMF):
                    if t == last_t:
                        nc.tensor.matmul(
                            ps_b[:],
                            s_t[:pc, t * nb : (t + 1) * nb],
                            x_t[:, jl * MMF : (jl + 1) * MMF],
                            start=(mm_b == 0),
                            stop=(mm_b == n_mmpc - 1),
                        )
                        mm_b += 1
                    else:
                        nc.tensor.matmul(
                            ps_a[:],
                            s_t[:pc, t * nb : (t + 1) * nb],
                            x_t[:, jl * MMF : (jl + 1) * MMF],
                            start=(mm_a == 0),
                            stop=(mm_a == mm_a_total - 1),
                        )
                        mm_a += 1
            combine_and_ship(o_tb, ps_b, nc.sync, out_b)

    nc.compile()
    return nc


_CACHED = {}


def _get_nc(gt, nb):
    key = (gt, nb)
    if key not in _CACHED:
        _CACHED[key] = build_kernel(gt, nb)
    return _CACHED[key]


def run(x, start_padding_indices, trace=False):
    eff, gt, nb, cores = plan_shards(start_padding_indices)
    in_maps, slot_maps = make_host_inputs(x, eff, gt, nb, cores)
    nc = _get_nc(gt, nb)
    res = bass_utils.run_bass_kernel_spmd(
        nc, in_maps, core_ids=list(range(N_CORES)), trace=trace
    )
    out_full = np.zeros((B, D), dtype=np.float32)
    t_chunks = -(-gt // CHUNK_G)
    for c in range(N_CORES):
        oc = res.results[c]["out_b"].reshape(nb, D)
        if t_chunks > 1:
            oc = oc + res.results[c]["out_a"].reshape(nb, D)
        for i, b in enumerate(slot_maps[c]):
            out_full[b] += oc[i]
    return out_full, res


def kernel(x, start_padding_indices):
    out, _ = run(x, start_padding_indices, trace=False)
    return out
